# revision 14
# baseline (speedup 1.0000x reference)
"""CCPL contrastive-loss kernel for Trainium2 (8 NeuronCores).

Strategy: the loss only touches 256 sampled 3x3 neighborhoods of
feat_q/feat_k (~4.7 MB of each 512 MiB tensor), so the kernel never
streams the full tensors.  Work is data-parallel over the batch dim:
core b receives feat_q[b] / feat_k[b] (64 MiB each staged to HBM) and a
program with the 256 sample windows baked in as static strided DMAs
(sample_ids are host-known at build time, identical for every core, so
the program is SPMD-clean).  Each core gathers [64c, 256s, 9] blocks for
q and k, normalizes over the channel dim, and emits one partial
sum(|q_hat - k_hat|); the host sums the 8 partials and divides by the
element count.
"""

import os
import sys
from contextlib import ExitStack

import numpy as np

sys.path.insert(0, "/opt/trn_rl_repo")

import concourse.bass as bass
import concourse.tile as tile
from concourse import mybir
from concourse.bass_utils import run_bass_kernel_spmd


def _install_ntff_hook():
    """Provide antenv.axon_hooks when the agent image lacks it.

    concourse's axon trace path imports antenv.axon_hooks to fetch the
    NTFF profile hook; this image's antenv has no such submodule.  The
    hook implementation ships in trn_agent_boot.trn_boot, so wire it up
    against the axon PJRT .so directly.
    """
    try:
        from antenv.axon_hooks import get_axon_ntff_profile_hook  # noqa: F401

        return
    except ImportError:
        pass
    import types

    hook = None
    try:
        from trn_agent_boot.trn_boot import _ntff_profile_via_ctypes

        so = "/opt/axon/libaxon_pjrt.so"
        if os.path.exists(so):
            hook = _ntff_profile_via_ctypes(so)
    except Exception:
        hook = None
    mod = types.ModuleType("antenv.axon_hooks")
    _state = {"hook": hook}
    mod.get_axon_ntff_profile_hook = lambda: _state["hook"]
    mod.set_axon_ntff_profile_hook = lambda h: _state.update(hook=h)
    import antenv

    sys.modules["antenv.axon_hooks"] = mod
    antenv.axon_hooks = mod


_install_ntff_hook()

B, C, H, W = 8, 64, 512, 512
NUM_S = 256
EPS = 1e-7
NCOL = NUM_S * 9  # 2304 columns: (sample, 3x3 window) with center at j=4
CHUNK = 384  # matmul moving-free <= 512; 6 even chunks
NCHUNK = NCOL // CHUNK
N_CORES = 8

_cache: dict = {}
LAST_RESULTS = None  # BassKernelResults of the most recent run (for test.py)


def _split_multi_waits(nc):
    """Walrus build here embeds at most ONE sync wait per instruction.

    Tile emits instructions (notably the kernel-tail Drain) carrying many
    sem waits.  Hoist all but the last wait of any such instruction onto
    single-wait NOPs inserted immediately before it on the same queue —
    the queue stalls on each NOP in turn, preserving semantics.
    """
    from concourse import mybir as _mybir

    for f in nc.m.functions:
        for blk in f.blocks:
            insts = blk.instructions
            i = 0
            while i < len(insts):
                inst = insts[i]
                si = inst.sync_info
                if si is not None and si.on_wait and len(si.on_wait) > 1:
                    waits = list(si.on_wait)
                    si.on_wait = waits[-1:]
                    for j, w in enumerate(waits[:-1]):
                        nop = _mybir.InstNoOp(
                            name=nc.get_next_instruction_name(),
                            ins=[],
                            outs=[],
                            engine=inst.engine,
                            sync_info=_mybir.SyncInfo(on_wait=[w], on_update=[]),
                        )
                        insts.insert(i + j, nop)
                    i += len(waits) - 1
                i += 1


def _build(ids):
    f32 = mybir.dt.float32
    nc = bass.Bass()
    fq = nc.dram_tensor("fq", [C, H, W], f32, kind="ExternalInput")
    fk = nc.dram_tensor("fk", [C, H, W], f32, kind="ExternalInput")
    out = nc.dram_tensor("out", [1, 1], f32, kind="ExternalOutput")

    with tile.TileContext(nc) as tc, ExitStack() as ctx:
        sb = ctx.enter_context(tc.tile_pool(name="sb", bufs=1))
        work = ctx.enter_context(tc.tile_pool(name="work", bufs=3))
        pn = ctx.enter_context(tc.tile_pool(name="pn", bufs=1, space="PSUM"))
        pb = ctx.enter_context(tc.tile_pool(name="pb", bufs=2, space="PSUM"))
        pf = ctx.enter_context(tc.tile_pool(name="pf", bufs=1, space="PSUM"))

        qraw = sb.tile([C, NUM_S, 9], f32)
        kraw = sb.tile([C, NUM_S, 9], f32)
        # Gather 3x3 windows: one strided DMA per (tensor, sample).
        # 12B contiguous runs x 3 rows x 64 channels per DMA; q on the SP
        # HWDGE ring, k on the ACT HWDGE ring so the two rings split the
        # descriptor-generation load.
        for s, (h, w) in enumerate(ids):
            nc.sync.dma_start(out=qraw[:, s, :], in_=fq[:, h : h + 3, w : w + 3])
            nc.scalar.dma_start(out=kraw[:, s, :], in_=fk[:, h : h + 3, w : w + 3])

        ones_col = sb.tile([C, 1], f32)
        nc.vector.memset(ones_col[:], 1.0)
        ones_row = sb.tile([1, C], f32)
        nc.vector.memset(ones_row[:], 1.0)

        # PE warmup: the matmul LDWEIGHTS struct fits only one embedded
        # sync wait, so advance PE's observed DVE clock past the memsets
        # here; the real matmuls then each need just one wait.
        warm = pf.tile([1, 1], f32, tag="warm")
        nc.tensor.matmul(
            out=warm[:], lhsT=ones_col[:], rhs=ones_col[:], start=True, stop=True
        )

        # d = window - center (center column j=4 becomes exactly 0).
        # Tile assigns DMA-completion sems round-robin per dma_start, so a
        # consumer of all 256 samples would need 4 lane waits — over the
        # ISA's embedded sync-wait budget.  Split by s mod 4 (= one lane
        # per class): each sub-instruction adds at most one new lane wait
        # to the vector engine's observed clock.
        dq = sb.tile([C, NUM_S, 9], f32)
        dk = sb.tile([C, NUM_S, 9], f32)
        nsub = NUM_S // 4
        for raw, d in ((qraw, dq), (kraw, dk)):
            for r in range(4):
                nc.vector.tensor_tensor(
                    out=d[:, r::4, :],
                    in0=raw[:, r::4, :],
                    in1=raw[:, r::4, 4:5].to_broadcast([C, nsub, 9]),
                    op=mybir.AluOpType.subtract,
                )
        dq2 = sb.tile([C, NUM_S, 9], f32)
        dk2 = sb.tile([C, NUM_S, 9], f32)
        nc.scalar.square(out=dq2[:], in_=dq[:])
        nc.scalar.square(out=dk2[:], in_=dk[:])

        dqf = dq[:].rearrange("p s n -> p (s n)")
        dkf = dk[:].rearrange("p s n -> p (s n)")
        dq2f = dq2[:].rearrange("p s n -> p (s n)")
        dk2f = dk2[:].rearrange("p s n -> p (s n)")

        # norm2[col] = sum_c d2[c, col]  (partition reduce via ones-matmul);
        # q occupies cols [0, NCOL), k occupies cols [NCOL, 2*NCOL) of one
        # partition-0 row so every matmul operand keeps base partition 0.
        norm = sb.tile([1, 2 * NCOL], f32)
        for i in range(NCHUNK):
            sl = slice(i * CHUNK, (i + 1) * CHUNK)
            slk = slice(NCOL + i * CHUNK, NCOL + (i + 1) * CHUNK)
            n2q = pn.tile([1, CHUNK], f32, tag="n2q")
            n2k = pn.tile([1, CHUNK], f32, tag="n2k")
            nc.tensor.matmul(
                out=n2q[:], lhsT=ones_col[:], rhs=dq2f[:, sl], start=True, stop=True
            )
            nc.tensor.matmul(
                out=n2k[:], lhsT=ones_col[:], rhs=dk2f[:, sl], start=True, stop=True
            )
            nc.scalar.sqrt(out=norm[:, sl], in_=n2q[:])
            nc.scalar.sqrt(out=norm[:, slk], in_=n2k[:])

        # rinv = 1 / (sqrt(norm2) + eps); center columns give 1/eps * 0 = 0
        rinv = sb.tile([1, 2 * NCOL], f32)
        nc.vector.tensor_scalar_add(out=norm[:], in0=norm[:], scalar1=EPS)
        nc.vector.reciprocal(out=rinv[:], in_=norm[:])

        # acc[c, i] = sum_cols_in_chunk |dq*rinv_q - dk*rinv_k|
        acc = sb.tile([C, NCHUNK], f32)
        for i in range(NCHUNK):
            sl = slice(i * CHUNK, (i + 1) * CHUNK)
            bq = pb.tile([C, CHUNK], f32)
            bk = pb.tile([C, CHUNK], f32)
            # broadcast rinv rows across 64 partitions via K=1 matmul
            nc.tensor.matmul(
                out=bq[:], lhsT=ones_row[:], rhs=rinv[:, sl], start=True, stop=True
            )
            nc.tensor.matmul(
                out=bk[:],
                lhsT=ones_row[:],
                rhs=rinv[:, NCOL + i * CHUNK : NCOL + (i + 1) * CHUNK],
                start=True,
                stop=True,
            )
            qh = work.tile([C, CHUNK], f32, tag="qh")
            kh = work.tile([C, CHUNK], f32, tag="kh")
            nc.vector.tensor_tensor(
                out=qh[:], in0=dqf[:, sl], in1=bq[:], op=mybir.AluOpType.mult
            )
            nc.vector.tensor_tensor(
                out=kh[:], in0=dkf[:, sl], in1=bk[:], op=mybir.AluOpType.mult
            )
            df = work.tile([C, CHUNK], f32, tag="df")
            nc.vector.tensor_tensor(
                out=df[:], in0=qh[:], in1=kh[:], op=mybir.AluOpType.subtract
            )
            nc.vector.tensor_reduce(
                out=acc[:, i : i + 1],
                in_=df[:],
                axis=mybir.AxisListType.X,
                op=mybir.AluOpType.add,
                apply_absolute_value=True,
            )

        accs = sb.tile([C, 1], f32)
        nc.vector.tensor_reduce(
            out=accs[:], in_=acc[:], axis=mybir.AxisListType.X, op=mybir.AluOpType.add
        )
        pfin = pf.tile([1, 1], f32, tag="fin")
        nc.tensor.matmul(
            out=pfin[:], lhsT=accs[:], rhs=ones_col[:], start=True, stop=True
        )
        res = sb.tile([1, 1], f32)
        nc.scalar.copy(out=res[:], in_=pfin[:])
        # SWDGE for the result write: the HWDGE lanes' completion sems were
        # all consumed by the 512 gathers, and a lane-reuse wait would push
        # this DMA over the 1-embedded-sync-wait ISA budget.
        nc.gpsimd.dma_start(out=out[:], in_=res[:])

    _split_multi_waits(nc)
    return nc


def kernel(feat_q, feat_k, sample_ids, *, trace=False, trace_cores=None):
    global LAST_RESULTS
    feat_q = np.ascontiguousarray(np.asarray(feat_q), dtype=np.float32)
    feat_k = np.ascontiguousarray(np.asarray(feat_k), dtype=np.float32)
    ids = np.asarray(sample_ids)
    ids_key = tuple(map(tuple, ids.astype(np.int64).tolist()))
    if ids_key not in _cache:
        _cache[ids_key] = _build(ids_key)
    nc = _cache[ids_key]

    in_maps = [
        {"fq": feat_q[b], "fk": feat_k[b]} for b in range(N_CORES)
    ]
    results = run_bass_kernel_spmd(
        nc,
        in_maps,
        core_ids=list(range(N_CORES)),
        trace=trace,
        trace_cores=trace_cores,
    )
    LAST_RESULTS = results
    total = np.float64(0.0)
    for r in results.results:
        total += np.float64(r["out"][0, 0])
    loss = total / (B * C * 8 * NUM_S)
    return np.asarray(loss, dtype=np.float32)


# revision 15
# speedup vs baseline: 1.4757x; 1.4757x over previous
"""CCPL contrastive-loss kernel for Trainium2 (8 NeuronCores).

Strategy: the loss only touches 256 sampled 3x3 neighborhoods of
feat_q/feat_k (~4.7 MB of each 512 MiB tensor), so the kernel never
streams the full tensors.  Work is data-parallel over the batch dim:
core b receives feat_q[b] / feat_k[b] (64 MiB each staged to HBM) and a
program with the 256 sample windows baked in as static strided DMAs
(sample_ids are host-known at build time, identical for every core, so
the program is SPMD-clean).  Each core gathers [64c, 256s, 9] blocks for
q and k, normalizes over the channel dim, and emits one partial
sum(|q_hat - k_hat|); the host sums the 8 partials and divides by the
element count.
"""

import os
import sys
from contextlib import ExitStack

import numpy as np

sys.path.insert(0, "/opt/trn_rl_repo")

import concourse.bass as bass
import concourse.tile as tile
from concourse import mybir
from concourse.bass_utils import run_bass_kernel_spmd


def _install_ntff_hook():
    """Provide antenv.axon_hooks when the agent image lacks it.

    concourse's axon trace path imports antenv.axon_hooks to fetch the
    NTFF profile hook; this image's antenv has no such submodule.  The
    hook implementation ships in trn_agent_boot.trn_boot, so wire it up
    against the axon PJRT .so directly.
    """
    try:
        from antenv.axon_hooks import get_axon_ntff_profile_hook  # noqa: F401

        return
    except ImportError:
        pass
    import types

    hook = None
    try:
        from trn_agent_boot.trn_boot import _ntff_profile_via_ctypes

        so = "/opt/axon/libaxon_pjrt.so"
        if os.path.exists(so):
            hook = _ntff_profile_via_ctypes(so)
    except Exception:
        hook = None
    mod = types.ModuleType("antenv.axon_hooks")
    _state = {"hook": hook}
    mod.get_axon_ntff_profile_hook = lambda: _state["hook"]
    mod.set_axon_ntff_profile_hook = lambda h: _state.update(hook=h)
    import antenv

    sys.modules["antenv.axon_hooks"] = mod
    antenv.axon_hooks = mod


_install_ntff_hook()

B, C, H, W = 8, 64, 512, 512
NUM_S = 256
EPS = 1e-7
NCOL = NUM_S * 9  # 2304 columns: (sample, 3x3 window) with center at j=4
CHUNK = 384  # matmul moving-free <= 512; 6 even chunks
NCHUNK = NCOL // CHUNK
N_CORES = 8

_cache: dict = {}
LAST_RESULTS = None  # BassKernelResults of the most recent run (for test.py)


def _split_multi_waits(nc):
    """Walrus build here embeds at most ONE sync wait per instruction.

    Tile emits instructions (notably the kernel-tail Drain) carrying many
    sem waits.  Hoist all but the last wait of any such instruction onto
    single-wait NOPs inserted immediately before it on the same queue —
    the queue stalls on each NOP in turn, preserving semantics.
    """
    from concourse import mybir as _mybir

    for f in nc.m.functions:
        for blk in f.blocks:
            insts = blk.instructions
            i = 0
            while i < len(insts):
                inst = insts[i]
                si = inst.sync_info
                if si is not None and si.on_wait and len(si.on_wait) > 1:
                    waits = list(si.on_wait)
                    si.on_wait = waits[-1:]
                    for j, w in enumerate(waits[:-1]):
                        nop = _mybir.InstNoOp(
                            name=nc.get_next_instruction_name(),
                            ins=[],
                            outs=[],
                            engine=inst.engine,
                            sync_info=_mybir.SyncInfo(on_wait=[w], on_update=[]),
                        )
                        insts.insert(i + j, nop)
                    i += len(waits) - 1
                i += 1


def _build(ids):
    f32 = mybir.dt.float32
    nc = bass.Bass()
    fq = nc.dram_tensor("fq", [C, H, W], f32, kind="ExternalInput")
    fk = nc.dram_tensor("fk", [C, H, W], f32, kind="ExternalInput")
    out = nc.dram_tensor("out", [1, 1], f32, kind="ExternalOutput")

    with tile.TileContext(nc) as tc, ExitStack() as ctx:
        sb = ctx.enter_context(tc.tile_pool(name="sb", bufs=1))
        work = ctx.enter_context(tc.tile_pool(name="work", bufs=3))
        pn = ctx.enter_context(tc.tile_pool(name="pn", bufs=1, space="PSUM"))
        pb = ctx.enter_context(tc.tile_pool(name="pb", bufs=2, space="PSUM"))
        pf = ctx.enter_context(tc.tile_pool(name="pf", bufs=1, space="PSUM"))

        ones_col = sb.tile([C, 1], f32)
        nc.vector.memset(ones_col[:], 1.0)
        ones_row = sb.tile([1, C], f32)
        nc.vector.memset(ones_row[:], 1.0)
        # PE warmup so later matmuls don't pay a fresh DVE-clock wait.
        warm = pf.tile([1, 1], f32, tag="warm")
        nc.tensor.matmul(
            out=warm[:], lhsT=ones_col[:], rhs=ones_col[:], start=True, stop=True
        )

        qraw = sb.tile([C, NUM_S, 9], f32)
        kraw = sb.tile([C, NUM_S, 9], f32)
        # Gather 3x3 windows: one strided DMA per (tensor, sample) with
        # 12B contiguous runs x 3 rows x 64 channels.  The bottleneck is
        # HWDGE descriptor generation (~3.5 ns/descriptor per ring), so
        # spread samples over all three descriptor generators: the SP and
        # ACT HWDGE rings plus the gpsimd SWDGE ring.
        qeng = [nc.sync, nc.scalar, nc.gpsimd]
        for s, (h, w) in enumerate(ids):
            eng = qeng[s % 3]
            eng.dma_start(out=qraw[:, s, :], in_=fq[:, h : h + 3, w : w + 3])
            eng.dma_start(out=kraw[:, s, :], in_=fk[:, h : h + 3, w : w + 3])

        # Process samples in groups so compute streams behind the gathers
        # instead of waiting for the final DMA.  GS*9 columns per group
        # keeps every matmul under the 512 moving-free limit.
        GS = 32  # samples per group
        GC = GS * 9  # 288 columns
        NG = NUM_S // GS  # 8 groups
        dq = sb.tile([C, NUM_S, 9], f32)
        dk = sb.tile([C, NUM_S, 9], f32)
        dq2 = sb.tile([C, NUM_S, 9], f32)
        dk2 = sb.tile([C, NUM_S, 9], f32)
        dqf = dq[:].rearrange("p s n -> p (s n)")
        dkf = dk[:].rearrange("p s n -> p (s n)")
        dq2f = dq2[:].rearrange("p s n -> p (s n)")
        dk2f = dk2[:].rearrange("p s n -> p (s n)")
        norm = sb.tile([1, 2 * NCOL], f32)
        rinv = sb.tile([1, 2 * NCOL], f32)
        acc = sb.tile([C, NG], f32)

        for g in range(NG):
            ss = slice(g * GS, (g + 1) * GS)
            sl = slice(g * GC, (g + 1) * GC)
            slk = slice(NCOL + g * GC, NCOL + (g + 1) * GC)
            # d = window - center (center column j=4 becomes exactly 0)
            nc.vector.tensor_tensor(
                out=dq[:, ss, :],
                in0=qraw[:, ss, :],
                in1=qraw[:, ss, 4:5].to_broadcast([C, GS, 9]),
                op=mybir.AluOpType.subtract,
            )
            nc.vector.tensor_tensor(
                out=dk[:, ss, :],
                in0=kraw[:, ss, :],
                in1=kraw[:, ss, 4:5].to_broadcast([C, GS, 9]),
                op=mybir.AluOpType.subtract,
            )
            nc.scalar.square(out=dq2[:, ss, :], in_=dq[:, ss, :])
            nc.scalar.square(out=dk2[:, ss, :], in_=dk[:, ss, :])
            # norm2[col] = sum_c d2[c, col] via ones-matmul (both matmul
            # operands keep base partition 0: q cols [0,NCOL), k offset by
            # NCOL in one partition-0 row)
            n2q = pn.tile([1, GC], f32, tag="n2q")
            n2k = pn.tile([1, GC], f32, tag="n2k")
            nc.tensor.matmul(
                out=n2q[:], lhsT=ones_col[:], rhs=dq2f[:, sl], start=True, stop=True
            )
            nc.tensor.matmul(
                out=n2k[:], lhsT=ones_col[:], rhs=dk2f[:, sl], start=True, stop=True
            )
            nc.scalar.sqrt(out=norm[:, sl], in_=n2q[:])
            nc.scalar.sqrt(out=norm[:, slk], in_=n2k[:])
            # rinv = 1/(sqrt(norm2)+eps); center cols give d*1/eps = 0
            nc.vector.tensor_scalar_add(
                out=norm[:, sl], in0=norm[:, sl], scalar1=EPS
            )
            nc.vector.tensor_scalar_add(
                out=norm[:, slk], in0=norm[:, slk], scalar1=EPS
            )
            nc.vector.reciprocal(out=rinv[:, sl], in_=norm[:, sl])
            nc.vector.reciprocal(out=rinv[:, slk], in_=norm[:, slk])
            # broadcast rinv across 64 partitions via K=1 matmul, apply,
            # and reduce |q_hat - k_hat| over the group's columns
            bq = pb.tile([C, GC], f32)
            bk = pb.tile([C, GC], f32)
            nc.tensor.matmul(
                out=bq[:], lhsT=ones_row[:], rhs=rinv[:, sl], start=True, stop=True
            )
            nc.tensor.matmul(
                out=bk[:], lhsT=ones_row[:], rhs=rinv[:, slk], start=True, stop=True
            )
            qh = work.tile([C, GC], f32, tag="qh")
            kh = work.tile([C, GC], f32, tag="kh")
            nc.vector.tensor_tensor(
                out=qh[:], in0=dqf[:, sl], in1=bq[:], op=mybir.AluOpType.mult
            )
            nc.vector.tensor_tensor(
                out=kh[:], in0=dkf[:, sl], in1=bk[:], op=mybir.AluOpType.mult
            )
            df = work.tile([C, GC], f32, tag="df")
            nc.vector.tensor_tensor(
                out=df[:], in0=qh[:], in1=kh[:], op=mybir.AluOpType.subtract
            )
            nc.vector.tensor_reduce(
                out=acc[:, g : g + 1],
                in_=df[:],
                axis=mybir.AxisListType.X,
                op=mybir.AluOpType.add,
                apply_absolute_value=True,
            )

        accs = sb.tile([C, 1], f32)
        nc.vector.tensor_reduce(
            out=accs[:], in_=acc[:], axis=mybir.AxisListType.X, op=mybir.AluOpType.add
        )
        pfin = pf.tile([1, 1], f32, tag="fin")
        nc.tensor.matmul(
            out=pfin[:], lhsT=accs[:], rhs=ones_col[:], start=True, stop=True
        )
        res = sb.tile([1, 1], f32)
        nc.scalar.copy(out=res[:], in_=pfin[:])
        nc.gpsimd.dma_start(out=out[:], in_=res[:])

    _split_multi_waits(nc)
    return nc


def kernel(feat_q, feat_k, sample_ids, *, trace=False, trace_cores=None):
    global LAST_RESULTS
    feat_q = np.ascontiguousarray(np.asarray(feat_q), dtype=np.float32)
    feat_k = np.ascontiguousarray(np.asarray(feat_k), dtype=np.float32)
    ids = np.asarray(sample_ids)
    ids_key = tuple(map(tuple, ids.astype(np.int64).tolist()))
    if ids_key not in _cache:
        _cache[ids_key] = _build(ids_key)
    nc = _cache[ids_key]

    in_maps = [
        {"fq": feat_q[b], "fk": feat_k[b]} for b in range(N_CORES)
    ]
    results = run_bass_kernel_spmd(
        nc,
        in_maps,
        core_ids=list(range(N_CORES)),
        trace=trace,
        trace_cores=trace_cores,
    )
    LAST_RESULTS = results
    total = np.float64(0.0)
    for r in results.results:
        total += np.float64(r["out"][0, 0])
    loss = total / (B * C * 8 * NUM_S)
    return np.asarray(loss, dtype=np.float32)


# revision 28
# speedup vs baseline: 1.6096x; 1.0907x over previous
"""CCPL contrastive-loss kernel for Trainium2 (8 NeuronCores).

Strategy: the loss only touches 256 sampled 3x3 neighborhoods of
feat_q/feat_k (~4.7 MB of each 512 MiB tensor), so the kernel never
streams the full tensors.  Work is data-parallel over the batch dim:
core b receives feat_q[b] / feat_k[b] (64 MiB each staged to HBM) and a
program with the 256 sample windows baked in as static strided DMAs
(sample_ids are host-known at build time, identical for every core, so
the program is SPMD-clean).  Each core gathers [64c, 256s, 9] blocks for
q and k, normalizes over the channel dim, and emits one partial
sum(|q_hat - k_hat|); the host sums the 8 partials and divides by the
element count.
"""

import os
import sys
from contextlib import ExitStack

import numpy as np

sys.path.insert(0, "/opt/trn_rl_repo")

import concourse.bass as bass
import concourse.tile as tile
from concourse import mybir
from concourse.bass_utils import run_bass_kernel_spmd


def _install_ntff_hook():
    """Provide antenv.axon_hooks when the agent image lacks it.

    concourse's axon trace path imports antenv.axon_hooks to fetch the
    NTFF profile hook; this image's antenv has no such submodule.  The
    hook implementation ships in trn_agent_boot.trn_boot, so wire it up
    against the axon PJRT .so directly.
    """
    try:
        from antenv.axon_hooks import get_axon_ntff_profile_hook  # noqa: F401

        return
    except ImportError:
        pass
    import types

    hook = None
    try:
        from trn_agent_boot.trn_boot import _ntff_profile_via_ctypes

        so = "/opt/axon/libaxon_pjrt.so"
        if os.path.exists(so):
            hook = _ntff_profile_via_ctypes(so)
    except Exception:
        hook = None
    mod = types.ModuleType("antenv.axon_hooks")
    _state = {"hook": hook}
    mod.get_axon_ntff_profile_hook = lambda: _state["hook"]
    mod.set_axon_ntff_profile_hook = lambda h: _state.update(hook=h)
    import antenv

    sys.modules["antenv.axon_hooks"] = mod
    antenv.axon_hooks = mod


_install_ntff_hook()

B, C, H, W = 8, 64, 512, 512
NUM_S = 256
EPS = 1e-7
NCOL = NUM_S * 9  # 2304 columns: (sample, 3x3 window) with center at j=4
CHUNK = 384  # matmul moving-free <= 512; 6 even chunks
NCHUNK = NCOL // CHUNK
N_CORES = 8

_cache: dict = {}
LAST_RESULTS = None  # BassKernelResults of the most recent run (for test.py)


def _split_multi_waits(nc):
    """Walrus build here embeds at most ONE sync wait per instruction.

    Tile emits instructions (notably the kernel-tail Drain) carrying many
    sem waits.  Hoist all but the last wait of any such instruction onto
    single-wait NOPs inserted immediately before it on the same queue —
    the queue stalls on each NOP in turn, preserving semantics.
    """
    from concourse import mybir as _mybir

    for f in nc.m.functions:
        for blk in f.blocks:
            insts = blk.instructions
            i = 0
            while i < len(insts):
                inst = insts[i]
                si = inst.sync_info
                if si is not None and si.on_wait and len(si.on_wait) > 1:
                    waits = list(si.on_wait)
                    si.on_wait = waits[-1:]
                    for j, w in enumerate(waits[:-1]):
                        nop = _mybir.InstNoOp(
                            name=nc.get_next_instruction_name(),
                            ins=[],
                            outs=[],
                            engine=inst.engine,
                            sync_info=_mybir.SyncInfo(on_wait=[w], on_update=[]),
                        )
                        insts.insert(i + j, nop)
                    i += len(waits) - 1
                i += 1


def _build(ids):
    f32 = mybir.dt.float32
    nc = bass.Bass()
    fq = nc.dram_tensor("fq", [C, H, W], f32, kind="ExternalInput")
    fk = nc.dram_tensor("fk", [C, H, W], f32, kind="ExternalInput")
    out = nc.dram_tensor("out", [1, 1], f32, kind="ExternalOutput")

    with tile.TileContext(nc) as tc, ExitStack() as ctx:
        sb = ctx.enter_context(tc.tile_pool(name="sb", bufs=1))
        work = ctx.enter_context(tc.tile_pool(name="work", bufs=3))
        pn = ctx.enter_context(tc.tile_pool(name="pn", bufs=1, space="PSUM"))
        pb = ctx.enter_context(tc.tile_pool(name="pb", bufs=2, space="PSUM"))
        pf = ctx.enter_context(tc.tile_pool(name="pf", bufs=1, space="PSUM"))

        ones_col = sb.tile([C, 1], f32)
        nc.vector.memset(ones_col[:], 1.0)
        ones_row = sb.tile([1, C], f32)
        nc.vector.memset(ones_row[:], 1.0)
        # PE warmup so later matmuls don't pay a fresh DVE-clock wait.
        warm = pf.tile([1, 1], f32, tag="warm")
        nc.tensor.matmul(
            out=warm[:], lhsT=ones_col[:], rhs=ones_col[:], start=True, stop=True
        )

        qraw = sb.tile([C, NUM_S, 9], f32)
        kraw = sb.tile([C, NUM_S, 9], f32)
        # Gather 3x3 windows: one strided DMA per (tensor, sample) with
        # 12B contiguous runs x 3 rows x 64 channels.  The bottleneck is
        # HWDGE descriptor generation (~3.5 ns/descriptor per ring), so
        # spread samples over all three descriptor generators: the SP and
        # ACT HWDGE rings plus the gpsimd SWDGE ring.
        qeng = [nc.sync, nc.scalar, nc.gpsimd]
        for s, (h, w) in enumerate(ids):
            eng = qeng[s % 3]
            eng.dma_start(out=qraw[:, s, :], in_=fq[:, h : h + 3, w : w + 3])
            eng.dma_start(out=kraw[:, s, :], in_=fk[:, h : h + 3, w : w + 3])

        # Process samples in groups so compute streams behind the gathers
        # instead of waiting for the final DMA.  GS*9 columns per group
        # keeps every matmul under the 512 moving-free limit.
        GS = 32  # samples per group
        GC = GS * 9  # 288 columns
        NG = NUM_S // GS  # 8 groups
        dq = sb.tile([C, NUM_S, 9], f32)
        dk = sb.tile([C, NUM_S, 9], f32)
        dq2 = sb.tile([C, NUM_S, 9], f32)
        dk2 = sb.tile([C, NUM_S, 9], f32)
        dqf = dq[:].rearrange("p s n -> p (s n)")
        dkf = dk[:].rearrange("p s n -> p (s n)")
        dq2f = dq2[:].rearrange("p s n -> p (s n)")
        dk2f = dk2[:].rearrange("p s n -> p (s n)")
        norm = sb.tile([1, 2 * NCOL], f32)
        rinv = sb.tile([1, 2 * NCOL], f32)
        acc = sb.tile([C, NG], f32)

        for g in range(NG):
            ss = slice(g * GS, (g + 1) * GS)
            sl = slice(g * GC, (g + 1) * GC)
            slk = slice(NCOL + g * GC, NCOL + (g + 1) * GC)
            # d = window - center (center column j=4 becomes exactly 0)
            nc.vector.tensor_tensor(
                out=dq[:, ss, :],
                in0=qraw[:, ss, :],
                in1=qraw[:, ss, 4:5].to_broadcast([C, GS, 9]),
                op=mybir.AluOpType.subtract,
            )
            nc.vector.tensor_tensor(
                out=dk[:, ss, :],
                in0=kraw[:, ss, :],
                in1=kraw[:, ss, 4:5].to_broadcast([C, GS, 9]),
                op=mybir.AluOpType.subtract,
            )
            nc.scalar.square(out=dq2[:, ss, :], in_=dq[:, ss, :])
            nc.scalar.square(out=dk2[:, ss, :], in_=dk[:, ss, :])
            # norm2[col] = sum_c d2[c, col] via ones-matmul (both matmul
            # operands keep base partition 0: q cols [0,NCOL), k offset by
            # NCOL in one partition-0 row)
            n2q = pn.tile([1, GC], f32, tag="n2q")
            n2k = pn.tile([1, GC], f32, tag="n2k")
            nc.tensor.matmul(
                out=n2q[:], lhsT=ones_col[:], rhs=dq2f[:, sl], start=True, stop=True
            )
            nc.tensor.matmul(
                out=n2k[:], lhsT=ones_col[:], rhs=dk2f[:, sl], start=True, stop=True
            )
            nc.scalar.sqrt(out=norm[:, sl], in_=n2q[:])
            nc.scalar.sqrt(out=norm[:, slk], in_=n2k[:])
            # rinv = 1/(sqrt(norm2)+eps); center cols give d*1/eps = 0
            nc.vector.tensor_scalar_add(
                out=norm[:, sl], in0=norm[:, sl], scalar1=EPS
            )
            nc.vector.tensor_scalar_add(
                out=norm[:, slk], in0=norm[:, slk], scalar1=EPS
            )
            nc.vector.reciprocal(out=rinv[:, sl], in_=norm[:, sl])
            nc.vector.reciprocal(out=rinv[:, slk], in_=norm[:, slk])
            # broadcast rinv across 64 partitions via K=1 matmul, apply,
            # and reduce |q_hat - k_hat| over the group's columns
            bq = pb.tile([C, GC], f32)
            bk = pb.tile([C, GC], f32)
            nc.tensor.matmul(
                out=bq[:], lhsT=ones_row[:], rhs=rinv[:, sl], start=True, stop=True
            )
            nc.tensor.matmul(
                out=bk[:], lhsT=ones_row[:], rhs=rinv[:, slk], start=True, stop=True
            )
            qh = work.tile([C, GC], f32, tag="qh")
            kh = work.tile([C, GC], f32, tag="kh")
            nc.vector.tensor_tensor(
                out=qh[:], in0=dqf[:, sl], in1=bq[:], op=mybir.AluOpType.mult
            )
            nc.vector.tensor_tensor(
                out=kh[:], in0=dkf[:, sl], in1=bk[:], op=mybir.AluOpType.mult
            )
            df = work.tile([C, GC], f32, tag="df")
            nc.vector.tensor_tensor(
                out=df[:], in0=qh[:], in1=kh[:], op=mybir.AluOpType.subtract
            )
            nc.vector.tensor_reduce(
                out=acc[:, g : g + 1],
                in_=df[:],
                axis=mybir.AxisListType.X,
                op=mybir.AluOpType.add,
                apply_absolute_value=True,
            )

        accs = sb.tile([C, 1], f32)
        nc.vector.tensor_reduce(
            out=accs[:], in_=acc[:], axis=mybir.AxisListType.X, op=mybir.AluOpType.add
        )
        pfin = pf.tile([1, 1], f32, tag="fin")
        nc.tensor.matmul(
            out=pfin[:], lhsT=accs[:], rhs=ones_col[:], start=True, stop=True
        )
        res = sb.tile([1, 1], f32)
        nc.scalar.copy(out=res[:], in_=pfin[:])
        nc.gpsimd.dma_start(out=out[:], in_=res[:])

    _split_multi_waits(nc)
    return nc


def kernel(feat_q, feat_k, sample_ids, *, trace=False, trace_cores=None):
    global LAST_RESULTS
    feat_q = np.ascontiguousarray(np.asarray(feat_q), dtype=np.float32)
    feat_k = np.ascontiguousarray(np.asarray(feat_k), dtype=np.float32)
    ids = np.asarray(sample_ids)
    ids_key = tuple(map(tuple, ids.astype(np.int64).tolist()))
    if ids_key not in _cache:
        _cache[ids_key] = _build(ids_key)
    nc = _cache[ids_key]

    in_maps = [
        {"fq": feat_q[b], "fk": feat_k[b]} for b in range(N_CORES)
    ]
    results = run_bass_kernel_spmd(
        nc,
        in_maps,
        core_ids=list(range(N_CORES)),
        trace=trace,
        trace_cores=trace_cores,
    )
    LAST_RESULTS = results
    total = np.float64(0.0)
    for r in results.results:
        total += np.float64(r["out"][0, 0])
    loss = total / (B * C * 8 * NUM_S)
    return np.asarray(loss, dtype=np.float32)
